# revision 4
# baseline (speedup 1.0000x reference)
"""Trainium2 Bass kernel for the CIN block:
out[b,o,k] = sum_{h,m} W[o, h*M+m] * xl[b,h,k] * x0[b,m,k] + bias[o]

Strategy (data-parallel over batch across 8 cores, 32 batches/core,
processed in 8 groups of 4 batches; all GEMM operands bf16, fp32 PSUM):
  - fmap chunk p (rows c=128p..128p+128, c=(h,m)) is built directly in
    [C, K] layout: a contraction-2 matmul broadcasts the two xl rows of
    the chunk into PSUM (4 chunks run concurrently via tile_position
    row-groups), then the chunk is multiplied by x0 (stacked twice along
    partitions, 4 batches along free). The multiply is spread across
    engines: DVE scalar_tensor_tensor straight from PSUM for some
    chunks, ScalarE-evacuate + DVE / GpSimd tensor_mul for the rest.
  - GEMM: lhsT = W^T chunks [128c, 128o] (stationary), rhs = fmap chunk
    [128c, 512] (4 batches of K=128), accumulated over 32 chunks into
    2 PSUM banks (O=256 -> 2 o-chunks).
  - Bias is added during PSUM evacuation via ScalarE activation.
"""

import sys
import types
import warnings

warnings.filterwarnings("ignore")

import numpy as np
import ml_dtypes

B, M, H, K, O = 256, 64, 64, 128, 256
C = H * M                  # 4096 channels
NCORES = 8
BPC = B // NCORES          # 32 batches per core
GRP = 4                    # batches per group (moving dim = GRP*K = 512)
NG = BPC // GRP            # 8 groups per core
KB = GRP * K               # 512
NCHUNK = C // 128          # 32 contraction chunks
NSUP = NCHUNK // 4         # 8 superchunks (4 row-packed broadcasts each)

_BF16 = ml_dtypes.bfloat16

LAST_EXEC_NS = None


def _install_ntff_hook():
    try:
        from antenv.axon_hooks import get_axon_ntff_profile_hook  # noqa: F401
        return
    except ImportError:
        pass
    try:
        from trn_agent_boot.trn_boot import _ntff_profile_via_ctypes
        hook = _ntff_profile_via_ctypes('/opt/axon/libaxon_pjrt.so')
    except Exception:
        hook = None
    m = types.ModuleType('antenv.axon_hooks')
    m.get_axon_ntff_profile_hook = lambda: hook
    m.set_axon_ntff_profile_hook = lambda h: None
    sys.modules['antenv.axon_hooks'] = m


_NC_CACHE = {}


def _consumer_kind(p):
    # Spread the per-chunk fmap multiply across engines.
    r = p % 8
    if r < 3:
        return "stt"       # DVE scalar_tensor_tensor straight from PSUM
    if r == 7:
        return "gp"        # ScalarE evac + GpSimd tensor_mul
    return "dve"           # ScalarE evac + DVE tensor_mul


def _build_program():
    if "nc" in _NC_CACHE:
        return _NC_CACHE["nc"]
    import concourse.bacc as bacc
    import concourse.tile as tile
    import concourse.mybir as mybir

    dt = mybir.dt
    nc = bacc.Bacc("TRN2", target_bir_lowering=False, debug=False)

    x0s_d = nc.dram_tensor("x0s", [NG, 128, KB], dt.bfloat16, kind="ExternalInput").ap()
    xlp_d = nc.dram_tensor("xlp", [NG, 128, NSUP * KB], dt.bfloat16, kind="ExternalInput").ap()
    wt_d = nc.dram_tensor("wt", [128, NCHUNK * O], dt.bfloat16, kind="ExternalInput").ap()
    e4_d = nc.dram_tensor("e4", [128, 128], dt.bfloat16, kind="ExternalInput").ap()
    bias_d = nc.dram_tensor("bias_t", [128, 2], dt.float32, kind="ExternalInput").ap()
    out_d = nc.dram_tensor("out", [BPC, O, K], dt.float32, kind="ExternalOutput").ap()

    with tile.TileContext(nc) as tc:
        with tc.tile_pool(name="const", bufs=1) as cpool, \
             tc.tile_pool(name="io", bufs=2) as iopool, \
             tc.tile_pool(name="fmapp", bufs=2) as fpool, \
             tc.tile_pool(name="xlbp", bufs=6) as xlbpool, \
             tc.tile_pool(name="outp", bufs=2) as opool, \
             tc.tile_pool(name="psx", bufs=4, space="PSUM") as psx, \
             tc.tile_pool(name="psg", bufs=2, space="PSUM") as psg:

            wt = cpool.tile([128, NCHUNK * O], dt.bfloat16)
            nc.sync.dma_start(wt[:], wt_d[:])
            e4 = cpool.tile([128, 128], dt.bfloat16)
            nc.sync.dma_start(e4[:], e4_d[:])
            bias_t = cpool.tile([128, 2], dt.float32)
            nc.sync.dma_start(bias_t[:], bias_d[:])

            for g in range(NG):
                x0s = iopool.tile([128, KB], dt.bfloat16, name=f"x0s_{g}", tag="x0s")
                nc.sync.dma_start(x0s[:], x0s_d[g])
                xlp = iopool.tile([128, NSUP * KB], dt.bfloat16, name=f"xlp_{g}", tag="xlp")
                nc.sync.dma_start(xlp[:], xlp_d[g])

                fmap = fpool.tile([128, NCHUNK * KB], dt.bfloat16, name=f"fmap_{g}", tag="fmap")
                for s in range(NSUP):
                    pss = []
                    for i in range(4):
                        p = 4 * s + i
                        ps_x = psx.tile([128, KB], dt.float32, name=f"psx_{g}_{p}", tag="psx")
                        nc.tensor.matmul(ps_x[:], e4[32 * i:32 * i + 2, :],
                                         xlp[32 * i:32 * i + 2, KB * s:KB * (s + 1)],
                                         start=True, stop=True, tile_position=(32 * i, 0))
                        pss.append(ps_x)
                    for i in range(4):
                        p = 4 * s + i
                        ps_x = pss[i]
                        kind = _consumer_kind(p)
                        dst = fmap[:, KB * p:KB * (p + 1)]
                        if kind == "stt":
                            nc.vector.scalar_tensor_tensor(
                                dst, ps_x[:], 1.0, x0s[:],
                                mybir.AluOpType.mult, mybir.AluOpType.mult)
                        else:
                            xlb = xlbpool.tile([128, KB], dt.bfloat16,
                                               name=f"xlb_{g}_{p}", tag="xlb")
                            nc.scalar.copy(xlb[:], ps_x[:])
                            if kind == "gp":
                                nc.gpsimd.tensor_mul(dst, xlb[:], x0s[:])
                            else:
                                nc.vector.tensor_mul(dst, xlb[:], x0s[:])

                pso = [psg.tile([128, KB], dt.float32, name=f"psg_{g}_{oc}", tag=f"psg{oc}")
                       for oc in range(2)]
                for p in range(NCHUNK):
                    for oc in range(2):
                        nc.tensor.matmul(pso[oc][:],
                                         wt[:, O * p + 128 * oc:O * p + 128 * (oc + 1)],
                                         fmap[:, KB * p:KB * (p + 1)],
                                         start=(p == 0), stop=(p == NCHUNK - 1))
                for oc in range(2):
                    osb = opool.tile([128, KB], dt.float32, name=f"osb_{g}_{oc}", tag=f"osb{oc}")
                    nc.scalar.activation(osb[:], pso[oc][:],
                                         mybir.ActivationFunctionType.Identity,
                                         bias=bias_t[:, oc:oc + 1])
                    dst = out_d[GRP * g:GRP * (g + 1), 128 * oc:128 * (oc + 1), :] \
                        .rearrange("g o k -> o g k")
                    nc.sync.dma_start(dst, osb[:, :].rearrange("o (g k) -> o g k", k=K))

    nc.compile()
    _NC_CACHE["nc"] = nc
    return nc


def _host_prep(x0, xl, W, b):
    # x0s[core][g]: [128, KB]  rows j = x0[b, j%64, :], cols gi*K+kk (b = 32c+4g+gi)
    x0g = x0.reshape(NCORES, NG, GRP, M, K).transpose(0, 1, 3, 2, 4) \
        .reshape(NCORES, NG, M, KB)
    x0s = np.concatenate([x0g, x0g], axis=2).astype(_BF16)  # [NC, NG, 128, KB]

    # xlp[core][g]: [128, NSUP*KB]; partition 32i+r (i=0..3, r=0..1) holds, at
    # free offset s*KB + gi*K + kk, the value xl[b(g,gi), 8s+2i+r, kk]
    # (chunk p = 4s+i uses xl rows {2p, 2p+1} = {8s+2i, 8s+2i+1}).
    arr = xl.reshape(NCORES, NG, GRP, NSUP, 4, 2, K).transpose(0, 1, 4, 5, 3, 2, 6)
    # arr: [NC, NG, i(4), r(2), s(8), gi(4), kk] -> rows packed per (i, r)
    arr = arr.reshape(NCORES, NG, 8, NSUP * KB)
    xlp = np.zeros((NCORES, NG, 128, NSUP * KB), dtype=np.float32)
    for i in range(4):
        for r in range(2):
            xlp[:, :, 32 * i + r, :] = arr[:, :, 2 * i + r, :]
    xlp = xlp.astype(_BF16)

    Wm = W[:, :, 0]                        # [O, C]
    wt = np.ascontiguousarray(Wm.T).reshape(NCHUNK, 128, O).transpose(1, 0, 2) \
        .reshape(128, NCHUNK * O).astype(_BF16)   # wt[j, p*O+o] = W[o, 128p+j]

    e4 = np.zeros((128, 128), dtype=np.float32)
    for i in range(4):
        e4[32 * i + 0, 0:64] = 1.0
        e4[32 * i + 1, 64:128] = 1.0
    e4 = e4.astype(_BF16)

    bias_t = np.ascontiguousarray(b.reshape(2, 128).T.astype(np.float32))  # [128, 2]
    return x0s, xlp, wt, e4, bias_t


def kernel(x0, xl, k, W, b, _trace=False):
    global LAST_EXEC_NS
    _install_ntff_hook()
    import concourse.bass_utils as bass_utils

    x0 = np.asarray(x0, dtype=np.float32)
    xl = np.asarray(xl, dtype=np.float32)
    W = np.asarray(W, dtype=np.float32)
    b = np.asarray(b, dtype=np.float32)

    nc = _build_program()
    x0s, xlp, wt, e4, bias_t = _host_prep(x0, xl, W, b)

    in_maps = [
        {"x0s": np.ascontiguousarray(x0s[c]), "xlp": np.ascontiguousarray(xlp[c]),
         "wt": wt, "e4": e4, "bias_t": bias_t}
        for c in range(NCORES)
    ]
    res = bass_utils.run_bass_kernel_spmd(
        nc, in_maps, core_ids=list(range(NCORES)), trace=_trace)
    LAST_EXEC_NS = res.exec_time_ns

    out = np.concatenate([res.results[c]["out"][None] for c in range(NCORES)], axis=0)
    return np.ascontiguousarray(out.reshape(B, O, K)).astype(np.float32)


# revision 5
# speedup vs baseline: 1.0425x; 1.0425x over previous
"""Trainium2 Bass kernel for the CIN block:
out[b,o,k] = sum_{h,m} W[o, h*M+m] * xl[b,h,k] * x0[b,m,k] + bias[o]

Strategy (data-parallel over batch across 8 cores, 32 batches/core,
processed in 8 groups of 4 batches; all GEMM operands bf16, fp32 PSUM):
  - fmap chunk p (rows c=128p..128p+128, c=(h,m)) is built directly in
    [C, K] layout: a contraction-2 matmul broadcasts the two xl rows of
    the chunk into PSUM (4 chunks run concurrently via tile_position
    row-groups), then the chunk is multiplied by x0 (stacked twice along
    partitions, 4 batches along free). The multiply is spread across
    engines: DVE scalar_tensor_tensor straight from PSUM for some
    chunks, ScalarE-evacuate + DVE / GpSimd tensor_mul for the rest.
  - GEMM: lhsT = W^T chunks [128c, 128o] (stationary), rhs = fmap chunk
    [128c, 512] (4 batches of K=128), accumulated over 32 chunks into
    2 PSUM banks (O=256 -> 2 o-chunks).
  - Bias is added during PSUM evacuation via ScalarE activation.
"""

import sys
import types
import warnings

warnings.filterwarnings("ignore")

import numpy as np
import ml_dtypes

B, M, H, K, O = 256, 64, 64, 128, 256
C = H * M                  # 4096 channels
NCORES = 8
BPC = B // NCORES          # 32 batches per core
GRP = 4                    # batches per group (moving dim = GRP*K = 512)
NG = BPC // GRP            # 8 groups per core
KB = GRP * K               # 512
NCHUNK = C // 128          # 32 contraction chunks
NSUP = NCHUNK // 4         # 8 superchunks (4 row-packed broadcasts each)

_BF16 = ml_dtypes.bfloat16

LAST_EXEC_NS = None


def _install_ntff_hook():
    try:
        from antenv.axon_hooks import get_axon_ntff_profile_hook  # noqa: F401
        return
    except ImportError:
        pass
    try:
        from trn_agent_boot.trn_boot import _ntff_profile_via_ctypes
        hook = _ntff_profile_via_ctypes('/opt/axon/libaxon_pjrt.so')
    except Exception:
        hook = None
    m = types.ModuleType('antenv.axon_hooks')
    m.get_axon_ntff_profile_hook = lambda: hook
    m.set_axon_ntff_profile_hook = lambda h: None
    sys.modules['antenv.axon_hooks'] = m


_NC_CACHE = {}


def _consumer_kind(p):
    # Spread the per-chunk fmap multiply across engines.
    r = p % 8
    if r < 3:
        return "stt"       # DVE scalar_tensor_tensor straight from PSUM
    if r == 7:
        return "gp"        # ScalarE evac + GpSimd tensor_mul
    return "dve"           # ScalarE evac + DVE tensor_mul


def _build_program():
    if "nc" in _NC_CACHE:
        return _NC_CACHE["nc"]
    import concourse.bacc as bacc
    import concourse.tile as tile
    import concourse.mybir as mybir

    dt = mybir.dt
    nc = bacc.Bacc("TRN2", target_bir_lowering=False, debug=False)

    x0s_d = nc.dram_tensor("x0s", [NG, 128, KB], dt.bfloat16, kind="ExternalInput").ap()
    xlp_d = nc.dram_tensor("xlp", [NG, 128, NSUP * KB], dt.bfloat16, kind="ExternalInput").ap()
    wt_d = nc.dram_tensor("wt", [128, NCHUNK * O], dt.bfloat16, kind="ExternalInput").ap()
    e4_d = nc.dram_tensor("e4", [128, 128], dt.bfloat16, kind="ExternalInput").ap()
    bias_d = nc.dram_tensor("bias_t", [128, 2], dt.float32, kind="ExternalInput").ap()
    out_d = nc.dram_tensor("out", [BPC, O, K], dt.float32, kind="ExternalOutput").ap()

    with tile.TileContext(nc) as tc:
        with tc.tile_pool(name="const", bufs=1) as cpool, \
             tc.tile_pool(name="io", bufs=2) as iopool, \
             tc.tile_pool(name="fmapp", bufs=2) as fpool, \
             tc.tile_pool(name="xlbp", bufs=6) as xlbpool, \
             tc.tile_pool(name="outp", bufs=2) as opool, \
             tc.tile_pool(name="psx", bufs=4, space="PSUM") as psx, \
             tc.tile_pool(name="psg", bufs=2, space="PSUM") as psg:

            wt = cpool.tile([128, NCHUNK * O], dt.bfloat16)
            nc.sync.dma_start(wt[:], wt_d[:])
            e4 = cpool.tile([128, 128], dt.bfloat16)
            nc.sync.dma_start(e4[:], e4_d[:])
            bias_t = cpool.tile([128, 2], dt.float32)
            nc.sync.dma_start(bias_t[:], bias_d[:])

            def emit_gemm(g, fmap):
                pso = [psg.tile([128, KB], dt.float32, name=f"psg_{g}_{oc}", tag=f"psg{oc}")
                       for oc in range(2)]
                for p in range(NCHUNK):
                    for oc in range(2):
                        nc.tensor.matmul(pso[oc][:],
                                         wt[:, O * p + 128 * oc:O * p + 128 * (oc + 1)],
                                         fmap[:, KB * p:KB * (p + 1)],
                                         start=(p == 0), stop=(p == NCHUNK - 1))
                for oc in range(2):
                    osb = opool.tile([128, KB], dt.float32, name=f"osb_{g}_{oc}", tag=f"osb{oc}")
                    nc.scalar.activation(osb[:], pso[oc][:],
                                         mybir.ActivationFunctionType.Identity,
                                         bias=bias_t[:, oc:oc + 1])
                    dst = out_d[GRP * g:GRP * (g + 1), 128 * oc:128 * (oc + 1), :] \
                        .rearrange("g o k -> o g k")
                    nc.sync.dma_start(dst, osb[:, :].rearrange("o (g k) -> o g k", k=K))

            prev = None
            for g in range(NG):
                x0s = iopool.tile([128, KB], dt.bfloat16, name=f"x0s_{g}", tag="x0s")
                nc.sync.dma_start(x0s[:], x0s_d[g])
                xlp = iopool.tile([128, NSUP * KB], dt.bfloat16, name=f"xlp_{g}", tag="xlp")
                nc.sync.dma_start(xlp[:], xlp_d[g])

                fmap = fpool.tile([128, NCHUNK * KB], dt.bfloat16, name=f"fmap_{g}", tag="fmap")
                for s in range(NSUP):
                    pss = []
                    for i in range(4):
                        p = 4 * s + i
                        ps_x = psx.tile([128, KB], dt.float32, name=f"psx_{g}_{p}", tag="psx")
                        nc.tensor.matmul(ps_x[:], e4[32 * i:32 * i + 2, :],
                                         xlp[32 * i:32 * i + 2, KB * s:KB * (s + 1)],
                                         start=True, stop=True, tile_position=(32 * i, 0))
                        pss.append(ps_x)
                    for i in range(4):
                        p = 4 * s + i
                        ps_x = pss[i]
                        kind = _consumer_kind(p)
                        dst = fmap[:, KB * p:KB * (p + 1)]
                        if kind == "stt":
                            nc.vector.scalar_tensor_tensor(
                                dst, ps_x[:], 1.0, x0s[:],
                                mybir.AluOpType.mult, mybir.AluOpType.mult)
                        else:
                            xlb = xlbpool.tile([128, KB], dt.bfloat16,
                                               name=f"xlb_{g}_{p}", tag="xlb")
                            nc.scalar.copy(xlb[:], ps_x[:])
                            if kind == "gp":
                                nc.gpsimd.tensor_mul(dst, xlb[:], x0s[:])
                            else:
                                nc.vector.tensor_mul(dst, xlb[:], x0s[:])

                if prev is not None:
                    emit_gemm(prev[0], prev[1])
                prev = (g, fmap)
            emit_gemm(prev[0], prev[1])

    nc.compile()
    _NC_CACHE["nc"] = nc
    return nc


def _host_prep(x0, xl, W, b):
    # x0s[core][g]: [128, KB]  rows j = x0[b, j%64, :], cols gi*K+kk (b = 32c+4g+gi)
    x0g = x0.reshape(NCORES, NG, GRP, M, K).transpose(0, 1, 3, 2, 4) \
        .reshape(NCORES, NG, M, KB)
    x0s = np.concatenate([x0g, x0g], axis=2).astype(_BF16)  # [NC, NG, 128, KB]

    # xlp[core][g]: [128, NSUP*KB]; partition 32i+r (i=0..3, r=0..1) holds, at
    # free offset s*KB + gi*K + kk, the value xl[b(g,gi), 8s+2i+r, kk]
    # (chunk p = 4s+i uses xl rows {2p, 2p+1} = {8s+2i, 8s+2i+1}).
    arr = xl.reshape(NCORES, NG, GRP, NSUP, 4, 2, K).transpose(0, 1, 4, 5, 3, 2, 6)
    # arr: [NC, NG, i(4), r(2), s(8), gi(4), kk] -> rows packed per (i, r)
    arr = arr.reshape(NCORES, NG, 8, NSUP * KB)
    xlp = np.zeros((NCORES, NG, 128, NSUP * KB), dtype=np.float32)
    for i in range(4):
        for r in range(2):
            xlp[:, :, 32 * i + r, :] = arr[:, :, 2 * i + r, :]
    xlp = xlp.astype(_BF16)

    Wm = W[:, :, 0]                        # [O, C]
    wt = np.ascontiguousarray(Wm.T).reshape(NCHUNK, 128, O).transpose(1, 0, 2) \
        .reshape(128, NCHUNK * O).astype(_BF16)   # wt[j, p*O+o] = W[o, 128p+j]

    e4 = np.zeros((128, 128), dtype=np.float32)
    for i in range(4):
        e4[32 * i + 0, 0:64] = 1.0
        e4[32 * i + 1, 64:128] = 1.0
    e4 = e4.astype(_BF16)

    bias_t = np.ascontiguousarray(b.reshape(2, 128).T.astype(np.float32))  # [128, 2]
    return x0s, xlp, wt, e4, bias_t


def kernel(x0, xl, k, W, b, _trace=False):
    global LAST_EXEC_NS
    _install_ntff_hook()
    import concourse.bass_utils as bass_utils

    x0 = np.asarray(x0, dtype=np.float32)
    xl = np.asarray(xl, dtype=np.float32)
    W = np.asarray(W, dtype=np.float32)
    b = np.asarray(b, dtype=np.float32)

    nc = _build_program()
    x0s, xlp, wt, e4, bias_t = _host_prep(x0, xl, W, b)

    in_maps = [
        {"x0s": np.ascontiguousarray(x0s[c]), "xlp": np.ascontiguousarray(xlp[c]),
         "wt": wt, "e4": e4, "bias_t": bias_t}
        for c in range(NCORES)
    ]
    res = bass_utils.run_bass_kernel_spmd(
        nc, in_maps, core_ids=list(range(NCORES)), trace=_trace)
    LAST_EXEC_NS = res.exec_time_ns

    out = np.concatenate([res.results[c]["out"][None] for c in range(NCORES)], axis=0)
    return np.ascontiguousarray(out.reshape(B, O, K)).astype(np.float32)
